# revision 19
# baseline (speedup 1.0000x reference)
"""TRN2 Bass kernel for nn_NMS (offset min-sum LDPC decoder, batch 256).

Self-contained: derives all index tables from the H input at call time,
shards the batch across 8 NeuronCores (32 per core), runs one SPMD Bass
program via run_bass_kernel_spmd, and gathers the full [256, 576] output.

Per-core layout: 128 partitions = 4 row-blocks x 32 batch; each row-block's
edges live on the free axis as [36 rows x 16 slots] (15 real + 1 pad).

v2 pipeline (per decoding iteration):
  X = gather(zrep) - E                 (Pool gather; DVE sub; fp32 X)
  min1 = segmin |X|                    (DVE reduce, abs fused)
  B3 = (A3==min1b ? BIG : A3)          (custom DVE op EXCL_BIG_ANT)
  min2 = segmin B3                     (DVE reduce)
  row sign parity via prefix-product   (Pool tensor_tensor_scan on sign(X))
  u12 = relu(al*minp - al*beta)        (ACT, fp16; same rounding as G3)
  G3  = relu(al*|X| - al*beta)         (ACT, fp16)
  w   = u1 + relu(u2 - G3)             (DVE TT + STT; exact argmin select
                                        because G3(min1)==u1, G3(|X|>=min2)>=u2)
  E   = w * (sign(X)*rowsign)          (fp16 packed TT, 2x)
  colsum via ONE permuted-run gather   (Pool) + 2 prefix adds (fp16 DVE)
  cross-block sum + 4x replicate       (PE one-hot fp16 matmul; depth>=4
                                        edges via tiny accumulate-matmuls)
  Z = colsum + r                       (DVE, fp32)
Columns are globally permuted (descending max-per-block degree) so the
colsum gather is one instruction with prefix-aligned depth runs; the host
permutes r on the way in and un-permutes the output.

Accuracy: X/Z/minima stay fp32 (branch decisions exact); E/w/u/colsum are
fp16 (measured end-to-end rel err ~1.3e-4 vs fp32 reference).

Multi-wait instructions are post-processed into standalone EventSemaphore
waits (hoist_waits) because this walrus build accepts only one sync-wait
slot per TPB instruction.
"""
import numpy as np
from contextlib import ExitStack

import concourse.bass as bass
import concourse.tile as tile
from concourse import mybir, library_config

FP32 = mybir.dt.float32
FP16 = mybir.dt.float16
U16 = mybir.dt.uint16

P = 128
B = 32           # batch per core
NBLK = 4
RPB = 36         # rows per block
KPAD = 16        # padded row degree
ROW_DEG = 15
EPB = RPB * KPAD  # 576 edge slots per block
N = 576          # columns
D_KEEP = 3       # depth runs gathered; deeper edges via tiny accum-matmuls
ITERS = 3
BIGX = np.float32(30000.0)   # pad value for X/zrep (fp16-safe after *alpha)
BIGEXCL = 1e30               # fp32 argmin-exclusion sentinel


# ------------------------------------------------- custom DVE op ----
def _register_excl():
    """Register EXCL_ABS_ANT: out = (|in0| == in1) ? s1 : |in0|, at runtime.

    in0 is signed X (abs fused, so the op doesn't wait on the ACT abs);
    in1 is a [P,S,N] stride-0 broadcast of the per-row min; s1 is a
    compile-time float (STT shape has no C2 slot)."""
    import concourse.dve_ops as mod
    from concourse.dve_spec import (Spec, Src0, Src1, C1, Zero, maxx, select,
                                    eq, lower)
    from concourse.dve_uop import DveOpSpec
    name = "EXCL_ABS_ANT"
    if name in mod._SUB_OPCODE_FOR_NAME:
        for o in mod.OPS:
            if o.name == name:
                return o
    row = mod._CUSTOM_DVE_ROW_BASE + len(mod.OPS)
    assert row < 0x20
    m = maxx(Src0, Zero - Src0)
    spec = Spec(
        body=select(eq(m, Src1), C1, m),
        reference=lambda in0, in1, s0, s1, imm2: np.where(
            np.abs(in0) == in1, np.float32(s1), np.abs(in0)
        ).astype(np.float32),
    )
    shas = {}
    for ver in ("v3", "v4"):
        s = DveOpSpec(name=name, opcode=row, uops=lower(spec, ver=ver),
                      rd1_en=True)
        shas[ver] = s.sha(ver)
    op = mod.DveOp(name, spec, subdim=False, uops_sha=shas)
    mod.OPS.append(op)
    mod._SUB_OPCODE_FOR_NAME[name] = row
    mod.CUSTOM_DVE_SPECS[name] = spec
    return op


EXCL_OP = _register_excl()


# ---------------------------------------------------------------- tables ----
def build_tables(H):
    MROWS = H.shape[0]
    cols = np.array([np.nonzero(H[m])[0] for m in range(MROWS)], dtype=np.int64)
    assert cols.shape == (MROWS, ROW_DEG)

    # ---- block assignment: minimize (depth>=4 edges, L2, L1) via convex
    # per-(block,col) penalty, delta-evaluated row swaps ----
    PEN = np.array([0.0, 0.0, 1.0, 60.0, 4000.0, 3e5, 2e7, 1e9, 1e9],
                   dtype=np.float64)

    def metrics(cnt):
        mx = cnt.max(axis=0)
        return (int(np.maximum(cnt - 3, 0).sum()), int((mx >= 3).sum()),
                int((mx >= 2).sum()), int(cnt.max()))

    best = None
    for restart in range(2):
        rs = np.random.default_rng(restart)
        perm = rs.permutation(MROWS)
        assign = np.zeros(MROWS, dtype=np.int64)
        sizes = [0] * NBLK
        cnt = np.zeros((NBLK, N), dtype=np.int32)
        for m in perm:
            bestj, bestpen = None, None
            for j in range(NBLK):
                if sizes[j] >= RPB:
                    continue
                p = PEN[cnt[j, cols[m]] + 1].sum()
                if bestpen is None or p < bestpen:
                    bestj, bestpen = j, p
            assign[m] = bestj
            sizes[bestj] += 1
            cnt[bestj, cols[m]] += 1
        for _sweep in range(40):
            improved = False
            for m1 in range(MROWS):
                for m2 in range(m1 + 1, MROWS):
                    j1, j2 = assign[m1], assign[m2]
                    if j1 == j2:
                        continue
                    c1, c2 = cols[m1], cols[m2]
                    cn1, cn2 = cnt[j1], cnt[j2]
                    aff1, aff2 = {}, {}
                    for c in c1:
                        aff1[c] = aff1.get(c, 0) - 1
                        aff2[c] = aff2.get(c, 0) + 1
                    for c in c2:
                        aff1[c] = aff1.get(c, 0) + 1
                        aff2[c] = aff2.get(c, 0) - 1
                    d = 0.0
                    for c, dd in aff1.items():
                        d += PEN[cn1[c] + dd] - PEN[cn1[c]]
                    for c, dd in aff2.items():
                        d += PEN[cn2[c] + dd] - PEN[cn2[c]]
                    if d < -1e-9:
                        for c, dd in aff1.items():
                            cn1[c] += dd
                        for c, dd in aff2.items():
                            cn2[c] += dd
                        assign[m1], assign[m2] = j2, j1
                        improved = True
            if not improved:
                break
        met = metrics(cnt)
        if best is None or met[:3] < best[0][:3]:
            best = (met, assign.copy(), cnt.copy())
    met, assign, cnt = best
    assert cnt.max() <= 4, f"block depth {cnt.max()} > 4"

    colidx = np.full((NBLK, RPB, KPAD), N, dtype=np.int64)
    rows_of_block = [np.array([m for m in range(MROWS) if assign[m] == j],
                              dtype=np.int64) for j in range(NBLK)]
    for j in range(NBLK):
        for mm, m in enumerate(rows_of_block[j]):
            colidx[j, mm, :ROW_DEG] = cols[m]

    # per-(block, col, depth) edge positions
    strip_pos = np.full((NBLK, N, 4), EPB, dtype=np.int64)
    fill = np.zeros((NBLK, N), dtype=np.int64)
    for j in range(NBLK):
        for mm in range(RPB):
            for k in range(ROW_DEG):
                n = colidx[j, mm, k]
                d = fill[j, n]
                fill[j, n] = d + 1
                strip_pos[j, n, d] = mm * KPAD + k

    # global column permutation: descending capped max-depth -> prefix runs
    mdeg = np.minimum(cnt, D_KEEP).max(axis=0)          # [N], 0..3
    porder = np.argsort(-mdeg, kind="stable").astype(np.int64)
    pos = np.zeros(N, dtype=np.int64)
    pos[porder] = np.arange(N)
    L1 = int((mdeg >= 2).sum())
    L2 = int((mdeg >= 3).sum())

    # overflow: per-block depth-3 edges (cnt==4) -> accumulate-matmuls
    overflow = [(j, int(pos[n]), int(strip_pos[j, n, 3]))
                for j in range(NBLK) for n in range(N)
                if strip_pos[j, n, 3] != EPB]
    assert len(overflow) <= 16, f"too many overflow edges: {len(overflow)}"

    # gather table: [run0: 576][run1: L1][run2: L2][pad to %16]
    T = 576 + L1 + L2
    TPAD = (T + 15) // 16 * 16
    gvals = []
    for j in range(NBLK):
        v = np.full(TPAD, EPB, dtype=np.int64)
        i = 0
        for d in range(D_KEEP):
            lim = [576, L1, L2][d]
            for p_ in range(lim):
                v[i] = strip_pos[j, porder[p_], d]
                i += 1
        gvals.append(v)

    zvals = []
    for j in range(NBLK):
        v = np.empty(EPB, dtype=np.int64)
        flat = colidx[j].reshape(-1)
        for i in range(EPB):
            v[i] = pos[flat[i]] if flat[i] < N else N
        zvals.append(v)

    def wrap(vals_per_block, num_idxs):
        t = np.zeros((P, num_idxs // 16), dtype=np.uint16)
        for c in range(8):
            j = c // 2
            v = vals_per_block[j]
            for i in range(num_idxs):
                t[16 * c + i % 16, i // 16] = v[i]
        return t

    zidx = wrap(zvals, EPB)
    gidx = wrap(gvals, TPAD)

    # one-hot cross-block sum + replicate: W[(j',b'), (j,b)] = (b'==b)
    wmat = np.zeros((P, P), dtype=np.float16)
    for jp in range(NBLK):
        for bp in range(B):
            for j in range(NBLK):
                wmat[jp * B + bp, j * B + bp] = 1.0
    return dict(zidx=zidx, gidx=gidx, wmat=wmat, colidx=colidx,
                porder=porder, L1=L1, L2=L2, TPAD=TPAD, overflow=overflow)


def build_x0(r_slice, colidx):
    """Host-side iteration-0 gather: x0[(j,b), (mm,k)] = r[b, col] (pads BIGX)."""
    rpad = np.concatenate([r_slice, np.full((B, 1), BIGX, np.float32)], axis=1)
    x0 = rpad[:, colidx]                      # [B, NBLK, RPB, KPAD]
    x0 = x0.transpose(1, 0, 2, 3).reshape(P, EPB)
    return np.ascontiguousarray(x0)


# ---------------------------------------------------------------- kernel ----
def hoist_waits(nc, max_embedded=1):
    """Split multi-wait instructions into standalone EventSemaphore waits."""
    k = 0
    for f in nc.m.functions:
        for b in f.blocks:
            insts = b.instructions
            out = []
            for i in insts:
                tname = type(i).__name__
                si = i.sync_info
                if (si is not None and tname != "InstEventSemaphore"
                        and len(si.on_wait) > max_embedded):
                    waits = list(si.on_wait)
                    keep = waits[:max_embedded]
                    for w in waits[max_embedded:]:
                        es = mybir.InstEventSemaphore(
                            name=f"hoistw{k}", ins=[], outs=[])
                        k += 1
                        es.engine = i.engine
                        es.sync_info = mybir.SyncInfo(on_wait=[w], on_update=[])
                        nc.inst_map[es.name] = es
                        out.append(es)
                    i.sync_info = mybir.SyncInfo(
                        on_wait=keep, on_update=list(si.on_update))
                out.append(i)
            b.instructions = out


def build_bass(alpha, beta, tables):
    """alpha/beta: lists of 3 floats (baked as immediates)."""
    L1, L2, TPAD = tables["L1"], tables["L2"], tables["TPAD"]
    overflow = tables["overflow"]
    nc = bass.Bass("TRN2", target_bir_lowering=False, debug=False)
    r_d = nc.dram_tensor("r", [B, N], FP32, kind="ExternalInput")
    x0_d = nc.dram_tensor("x0", [P, EPB], FP32, kind="ExternalInput")
    zidx_d = nc.dram_tensor("zidx", [P, EPB // 16], U16, kind="ExternalInput")
    gidx_d = nc.dram_tensor("gidx", [P, TPAD // 16], U16, kind="ExternalInput")
    wmat_d = nc.dram_tensor("wmat", [P, P], FP16, kind="ExternalInput")
    out_d = nc.dram_tensor("out", [B, N], FP32, kind="ExternalOutput")
    HALF = N // 2

    with tile.TileContext(nc) as tc:
        with ExitStack() as ctx:
            pool = ctx.enter_context(tc.tile_pool(name="main", bufs=1))
            pspool = ctx.enter_context(tc.tile_pool(name="ps", bufs=1, space="PSUM"))

            r_rep = pool.tile([P, N], FP32)
            zrep = pool.tile([P, N + 2], FP32)    # col N = BIGX pad
            zfin = pool.tile([P, N], FP32)
            X = pool.tile([P, EPB], FP32)
            Xg = pool.tile([P, EPB], FP32)
            A3 = pool.tile([P, EPB], FP32)
            B3 = pool.tile([P, EPB], FP32)
            E = pool.tile([P, EPB + 2], FP16)     # col EPB = zero slot
            sgn = pool.tile([P, EPB], FP16)
            S = pool.tile([P, EPB], FP16)         # sign prefix products
            rsb = pool.tile([P, RPB + 1], FP16)   # col 0 = 1.0
            rs = pool.tile([P, RPB], FP16)
            sgf = pool.tile([P, EPB], FP16)       # sign(X)*rowsign
            G3 = pool.tile([P, EPB], FP16)
            t1 = pool.tile([P, EPB], FP16)
            w = pool.tile([P, EPB], FP16)
            negb = pool.tile([P, EPB], FP16)      # scan op1 filler (-1)
            minp = pool.tile([P, 2, RPB], FP32)
            u12 = pool.tile([P, 2, RPB], FP16)
            u1f = pool.tile([P, RPB], FP16)
            G = pool.tile([P, TPAD], FP16)
            zidx = pool.tile([P, EPB // 16], U16)
            gidx = pool.tile([P, TPAD // 16], U16)
            wmat = pool.tile([P, P], FP16)
            biasc = pool.tile([P, ITERS], FP32)   # -alpha*beta per iter
            # one full bank per quarter so matmul groups don't share banks
            zpsb = [pspool.tile([P, 512], FP32, name=f"zps{q}")
                    for q in range(4)]

            # ---- static loads (x0/r on the HWDGE queues first; index
            # tables + wmat via the Pool SWDGE path, which is idle early) ----
            r_bc = bass.AP(tensor=r_d.ap().tensor, offset=0,
                           ap=[[0, NBLK], [N, B], [1, N]])
            nc.sync.dma_start(X[:], x0_d[:])
            nc.scalar.dma_start(r_rep[:], r_bc)
            nc.gpsimd.dma_start(zidx[:], zidx_d[:])
            nc.gpsimd.dma_start(gidx[:], gidx_d[:])
            nc.gpsimd.dma_start(wmat[:], wmat_d[:])
            nc.vector.memset(zrep[:, N:N + 2], float(BIGX))
            nc.vector.memset(E[:, EPB:EPB + 2], 0.0)
            nc.vector.memset(rsb[:, 0:1], 1.0)
            nc.vector.memset(negb[:], -1.0)
            for it in range(ITERS):
                nc.vector.memset(biasc[:, it:it + 1],
                                 -float(alpha[it]) * float(beta[it]))
            # consume the index-table DMA deps early + warm PE path
            idxtouch = pool.tile([P, 2], U16)
            nc.gpsimd.tensor_copy(idxtouch[:, 0:1], zidx[:, 0:1])
            nc.gpsimd.tensor_copy(idxtouch[:, 1:2], gidx[:, 0:1])
            nc.tensor.matmul(zpsb[0][0:1, 0:1], lhsT=wmat[0:B, 0:1],
                             rhs=wmat[0:B, 0:1], start=True, stop=True)
            nc.vector.memset(w[:], 0.0)   # pad slots stay 0 forever

            def seg(t):   # [P, EPB] tile or AP -> [P, RPB, KPAD]
                ap = t if isinstance(t, bass.AP) else t[:]
                return ap.rearrange("p (a b) -> p a b", a=RPB)

            def rb(v):    # [P, RPB] row vec -> [P, RPB, KPAD] broadcast
                return v.unsqueeze(2).broadcast_to([P, RPB, KPAD])

            # overflow edges grouped by quarter (q = pos//144)
            ovf_by_q = ([], [], [], [])
            for (j0, p0, e0) in overflow:
                ovf_by_q[p0 // 144].append((j0, p0, e0))

            for it in range(ITERS):
                al = float(alpha[it])
                bias_ap = biasc[:, it:it + 1]
                last = it == ITERS - 1

                # ---- X = gather(zrep) - E  (it 0: host-precomputed X0) ----
                if it > 0:
                    nc.gpsimd.indirect_copy(Xg[:], zrep[:, 0:N + 1], zidx[:], True)
                    # strided over the 15 real slots; X pads keep BIGX from x0
                    nc.vector.tensor_sub(seg(X)[:, :, 0:ROW_DEG],
                                         seg(Xg)[:, :, 0:ROW_DEG],
                                         seg(E[:, 0:EPB])[:, :, 0:ROW_DEG])
                # sgn first: it heads the longest cross-engine chain
                # (ACT sgn -> Pool scan -> rowsign -> sgn_eff -> E)
                nc.scalar.activation(sgn[:], X[:],
                                     func=mybir.ActivationFunctionType.Sign)
                nc.vector.tensor_reduce(minp[:, 0, :], seg(X)[:, :, 0:ROW_DEG],
                                        axis=mybir.AxisListType.X,
                                        op=mybir.AluOpType.min,
                                        apply_absolute_value=True)
                nc.scalar.activation(A3[:], X[:],
                                     func=mybir.ActivationFunctionType.Abs)
                nc.vector._custom_dve(EXCL_OP, out=seg(B3)[:, :, 0:ROW_DEG],
                                      in0=seg(X)[:, :, 0:ROW_DEG],
                                      in1=minp[:, 0, :].unsqueeze(2)
                                      .broadcast_to([P, RPB, ROW_DEG]),
                                      s1=BIGEXCL)
                nc.vector.tensor_reduce(minp[:, 1, :], seg(B3)[:, :, 0:ROW_DEG],
                                        axis=mybir.AxisListType.X,
                                        op=mybir.AluOpType.min)
                nc.scalar.activation(G3[:], A3[:],
                                     func=mybir.ActivationFunctionType.Relu,
                                     scale=al, bias=bias_ap)
                # u rows on DVE (short hop after min2): u12raw = al*m - al*b,
                # u1fix = relu(u12raw[0]).  d6 uses u2 *raw* (g >= relu(u2raw)
                # for non-argmin edges, so the select stays exact).
                nc.vector.tensor_scalar(u12[:], minp[:], al, al * float(beta[it]),
                                        op0=mybir.AluOpType.mult,
                                        op1=mybir.AluOpType.subtract)
                nc.vector.tensor_scalar(u1f[:], u12[:, 0, :], 0.0, 0.0,
                                        op0=mybir.AluOpType.max,
                                        op1=mybir.AluOpType.add)

                # ---- row sign parity on Pool ----
                nc.gpsimd.tensor_tensor_scan(S[:], sgn[:], negb[:], 1.0,
                                             op0=mybir.AluOpType.mult,
                                             op1=mybir.AluOpType.max)
                ends = seg(S)[:, :, 15:16]                  # [P, RPB, 1]
                nc.gpsimd.tensor_copy(rsb[:, 1:RPB + 1], ends)
                nc.gpsimd.tensor_mul(rs[:], rsb[:, 1:RPB + 1], rsb[:, 0:RPB])
                nc.gpsimd.tensor_mul(seg(sgf), seg(sgn), rb(rs[:]))

                # ---- w = u1 + relu(u2 - G3); E = w * sgf ----
                nc.vector.tensor_sub(seg(t1)[:, :, 0:ROW_DEG],
                                     u12[:, 1, :].unsqueeze(2)
                                     .broadcast_to([P, RPB, ROW_DEG]),
                                     seg(G3)[:, :, 0:ROW_DEG])
                nc.vector.scalar_tensor_tensor(seg(w)[:, :, 0:ROW_DEG],
                                               seg(t1)[:, :, 0:ROW_DEG], 0.0,
                                               u1f[:].unsqueeze(2)
                                               .broadcast_to([P, RPB, ROW_DEG]),
                                               op0=mybir.AluOpType.max,
                                               op1=mybir.AluOpType.add)
                nc.vector.tensor_mul(E[:, 0:EPB], w[:], sgf[:])

                # ---- colsum: one gather + prefix adds + PE + z ----
                nc.gpsimd.indirect_copy(G[:], E[:, 0:EPB + 1], gidx[:], True)
                # add1 split so PE quarters start early (pi is depth-sorted)
                if L1 > HALF:
                    nc.vector.tensor_add(G[:, HALF:L1], G[:, HALF:L1],
                                         G[:, 576 + HALF:576 + L1])
                    nc.vector.tensor_add(G[:, 0:HALF], G[:, 0:HALF],
                                         G[:, 576:576 + HALF])
                else:
                    nc.vector.tensor_add(G[:, 0:L1], G[:, 0:L1],
                                         G[:, 576:576 + L1])
                if L2 > 0:
                    nc.vector.tensor_add(G[:, 0:L2], G[:, 0:L2],
                                         G[:, 576 + L1:576 + L1 + L2])

                # PE: 4 independent quarter groups (q3 first: clean region),
                # per-quarter z-add pipelined right after each group's stop
                for q in (3, 2, 1, 0):
                    ovf = ovf_by_q[q]
                    nc.tensor.matmul(zpsb[q][:, 0:144], lhsT=wmat[:],
                                     rhs=G[:, q * 144:q * 144 + 144],
                                     start=True, stop=(len(ovf) == 0))
                    for i, (j0, p0, e0) in enumerate(ovf):
                        nn = p0 - q * 144
                        nc.tensor.matmul(zpsb[q][:, nn:nn + 1],
                                         lhsT=wmat[32 * j0:32 * (j0 + 1), :],
                                         rhs=E[32 * j0:32 * (j0 + 1), e0:e0 + 1],
                                         start=False, stop=(i == len(ovf) - 1),
                                         tile_position=(32 * j0, 0))
                    sl = slice(q * 144, (q + 1) * 144)
                    if not last:
                        nc.vector.tensor_add(zrep[:, sl], zpsb[q][:, 0:144],
                                             r_rep[:, sl])
                    else:
                        nc.vector.tensor_add(zfin[0:B, sl],
                                             zpsb[q][0:B, 0:144],
                                             r_rep[0:B, sl])
            nc.sync.dma_start(out_d[:], zfin[0:B, :])

    hoist_waits(nc)
    return nc


# ------------------------------------------------------------ host driver ----
_CACHE = {}


def kernel(r, H, alpha, beta):
    r = np.asarray(r, dtype=np.float32)
    H = np.asarray(H, dtype=np.float32)
    alpha_l = [float(x) for x in np.asarray(alpha).reshape(-1)]
    beta_l = [float(x) for x in np.asarray(beta).reshape(-1)]

    key = (H.tobytes(), tuple(alpha_l), tuple(beta_l))
    if key not in _CACHE:
        tables = build_tables(H)
        nc = build_bass(alpha_l, beta_l, tables)
        _CACHE[key] = (tables, nc)
    tables, nc = _CACHE[key]
    porder = tables["porder"]

    from concourse.bass_utils import run_bass_kernel_spmd
    in_maps = []
    for c in range(8):
        rs_ = np.ascontiguousarray(r[c * B:(c + 1) * B])
        in_maps.append({
            "r": np.ascontiguousarray(rs_[:, porder]),
            "x0": build_x0(rs_, tables["colidx"]),
            "zidx": tables["zidx"],
            "gidx": tables["gidx"],
            "wmat": tables["wmat"],
        })
    # the first execution on a freshly-attached device occasionally fails
    # with NRT_EXEC_UNIT_UNRECOVERABLE; a retry succeeds
    last = None
    for _attempt in range(3):
        try:
            res = run_bass_kernel_spmd(nc, in_maps, core_ids=list(range(8)))
            break
        except Exception as e:  # noqa: BLE001
            last = e
    else:
        raise last
    out_p = np.concatenate([res.results[c]["out"] for c in range(8)], axis=0)
    out = np.empty_like(out_p)
    out[:, porder] = out_p
    return out.astype(np.float32)


# iteration 0 skips the X-gather: X comes from the x0 DMA issued at startup.
# The gather at the END of iterations 0 and 1 prepares the next X.


# revision 25
# speedup vs baseline: 1.0054x; 1.0054x over previous
"""TRN2 Bass kernel for nn_NMS (offset min-sum LDPC decoder, batch 256).

Self-contained: derives all index tables from the H input at call time,
shards the batch across 8 NeuronCores (32 per core), runs one SPMD Bass
program via run_bass_kernel_spmd, and gathers the full [256, 576] output.

Per-core layout: 128 partitions = 4 row-blocks x 32 batch; each row-block's
edges live on the free axis as [36 rows x 16 slots] (15 real + 1 pad).

v3 pipeline (per decoding iteration):
  X = gather(zrep) - E                 (Pool gather; DVE sub; fp32 X)
  A3 = |X|                             (DVE stt: (X*-1) max X)
  exclude-self row min via two scans   (DVE tensor_tensor_scan, op0=min
                                        op1=max; a BIG in data1 at each
                                        row-pad slot resets the running min,
                                        so no min1/min2/argmin machinery)
    Emin = min(prefix_excl, suffix_excl)   (DVE TT min of shifted views;
                                        suffix scan runs on reversed APs)
  row sign parity via prefix-product   (Pool tensor_tensor_scan on sign(X))
  G3e = relu(al*Emin - al*beta)        (ACT, fp16)
  E   = G3e * (sign(X)*rowsign)        (fp16 packed TT, 2x)
  colsum via ONE permuted-run gather   (Pool) + 2 prefix adds (fp16 DVE)
  cross-block sum + 4x replicate       (PE one-hot fp16 matmul; depth>=4
                                        edges via tiny accumulate-matmuls)
  Z = colsum + r                       (DVE, fp32)
Columns are globally permuted (descending max-per-block degree) so the
colsum gather is one instruction with prefix-aligned depth runs; the host
permutes r on the way in and un-permutes the output.

Accuracy: X/Z/Emin stay fp32 (the exclude-min is exact - min has no
rounding); E/colsum are fp16 (measured end-to-end rel err ~1e-4 vs the
fp32 reference).

Multi-wait instructions are post-processed into standalone EventSemaphore
waits (hoist_waits) because this walrus build accepts only one sync-wait
slot per TPB instruction.
"""
import numpy as np
from contextlib import ExitStack

import concourse.bass as bass
import concourse.tile as tile
from concourse import mybir, library_config

FP32 = mybir.dt.float32
FP16 = mybir.dt.float16
U16 = mybir.dt.uint16

P = 128
B = 32           # batch per core
NBLK = 4
RPB = 36         # rows per block
KPAD = 16        # padded row degree
ROW_DEG = 15
EPB = RPB * KPAD  # 576 edge slots per block
N = 576          # columns
D_KEEP = 3       # depth runs gathered; deeper edges via tiny accum-matmuls
ITERS = 3
BIGX = np.float32(30000.0)   # pad value for X/zrep (fp16-safe after *alpha)


# ---------------------------------------------------------------- tables ----
def build_tables(H):
    MROWS = H.shape[0]
    cols = np.array([np.nonzero(H[m])[0] for m in range(MROWS)], dtype=np.int64)
    assert cols.shape == (MROWS, ROW_DEG)

    # ---- block assignment: minimize (depth>=4 edges, L2, L1) via convex
    # per-(block,col) penalty, delta-evaluated row swaps ----
    PEN = np.array([0.0, 0.0, 1.0, 60.0, 4000.0, 3e5, 2e7, 1e9, 1e9],
                   dtype=np.float64)

    def metrics(cnt):
        mx = cnt.max(axis=0)
        return (int(np.maximum(cnt - 3, 0).sum()), int((mx >= 3).sum()),
                int((mx >= 2).sum()), int(cnt.max()))

    best = None
    for restart in range(2):
        rs = np.random.default_rng(restart)
        perm = rs.permutation(MROWS)
        assign = np.zeros(MROWS, dtype=np.int64)
        sizes = [0] * NBLK
        cnt = np.zeros((NBLK, N), dtype=np.int32)
        for m in perm:
            bestj, bestpen = None, None
            for j in range(NBLK):
                if sizes[j] >= RPB:
                    continue
                p = PEN[cnt[j, cols[m]] + 1].sum()
                if bestpen is None or p < bestpen:
                    bestj, bestpen = j, p
            assign[m] = bestj
            sizes[bestj] += 1
            cnt[bestj, cols[m]] += 1
        for _sweep in range(40):
            improved = False
            for m1 in range(MROWS):
                for m2 in range(m1 + 1, MROWS):
                    j1, j2 = assign[m1], assign[m2]
                    if j1 == j2:
                        continue
                    c1, c2 = cols[m1], cols[m2]
                    cn1, cn2 = cnt[j1], cnt[j2]
                    aff1, aff2 = {}, {}
                    for c in c1:
                        aff1[c] = aff1.get(c, 0) - 1
                        aff2[c] = aff2.get(c, 0) + 1
                    for c in c2:
                        aff1[c] = aff1.get(c, 0) + 1
                        aff2[c] = aff2.get(c, 0) - 1
                    d = 0.0
                    for c, dd in aff1.items():
                        d += PEN[cn1[c] + dd] - PEN[cn1[c]]
                    for c, dd in aff2.items():
                        d += PEN[cn2[c] + dd] - PEN[cn2[c]]
                    if d < -1e-9:
                        for c, dd in aff1.items():
                            cn1[c] += dd
                        for c, dd in aff2.items():
                            cn2[c] += dd
                        assign[m1], assign[m2] = j2, j1
                        improved = True
            if not improved:
                break
        met = metrics(cnt)
        if best is None or met[:3] < best[0][:3]:
            best = (met, assign.copy(), cnt.copy())
    met, assign, cnt = best
    assert cnt.max() <= 4, f"block depth {cnt.max()} > 4"

    colidx = np.full((NBLK, RPB, KPAD), N, dtype=np.int64)
    rows_of_block = [np.array([m for m in range(MROWS) if assign[m] == j],
                              dtype=np.int64) for j in range(NBLK)]
    for j in range(NBLK):
        for mm, m in enumerate(rows_of_block[j]):
            colidx[j, mm, :ROW_DEG] = cols[m]

    # per-(block, col, depth) edge positions
    strip_pos = np.full((NBLK, N, 4), EPB, dtype=np.int64)
    fill = np.zeros((NBLK, N), dtype=np.int64)
    for j in range(NBLK):
        for mm in range(RPB):
            for k in range(ROW_DEG):
                n = colidx[j, mm, k]
                d = fill[j, n]
                fill[j, n] = d + 1
                strip_pos[j, n, d] = mm * KPAD + k

    # global column permutation: descending capped max-depth -> prefix runs
    mdeg = np.minimum(cnt, D_KEEP).max(axis=0)          # [N], 0..3
    porder = np.argsort(-mdeg, kind="stable").astype(np.int64)
    pos = np.zeros(N, dtype=np.int64)
    pos[porder] = np.arange(N)
    L1 = int((mdeg >= 2).sum())
    L2 = int((mdeg >= 3).sum())

    # overflow: per-block depth-3 edges (cnt==4) -> accumulate-matmuls
    overflow = [(j, int(pos[n]), int(strip_pos[j, n, 3]))
                for j in range(NBLK) for n in range(N)
                if strip_pos[j, n, 3] != EPB]
    assert len(overflow) <= 16, f"too many overflow edges: {len(overflow)}"

    # gather table: [run0: 576][run1: L1][run2: L2][pad to %16]
    T = 576 + L1 + L2
    TPAD = (T + 15) // 16 * 16
    gvals = []
    for j in range(NBLK):
        v = np.full(TPAD, EPB, dtype=np.int64)
        i = 0
        for d in range(D_KEEP):
            lim = [576, L1, L2][d]
            for p_ in range(lim):
                v[i] = strip_pos[j, porder[p_], d]
                i += 1
        gvals.append(v)

    zvals = []
    for j in range(NBLK):
        v = np.empty(EPB, dtype=np.int64)
        flat = colidx[j].reshape(-1)
        for i in range(EPB):
            v[i] = pos[flat[i]] if flat[i] < N else N
        zvals.append(v)

    def wrap(vals_per_block, num_idxs):
        t = np.zeros((P, num_idxs // 16), dtype=np.uint16)
        for c in range(8):
            j = c // 2
            v = vals_per_block[j]
            for i in range(num_idxs):
                t[16 * c + i % 16, i // 16] = v[i]
        return t

    zidx = wrap(zvals, EPB)
    gidx = wrap(gvals, TPAD)

    # one-hot cross-block sum + replicate: W[(j',b'), (j,b)] = (b'==b)
    wmat = np.zeros((P, P), dtype=np.float16)
    for jp in range(NBLK):
        for bp in range(B):
            for j in range(NBLK):
                wmat[jp * B + bp, j * B + bp] = 1.0
    return dict(zidx=zidx, gidx=gidx, wmat=wmat, colidx=colidx,
                porder=porder, L1=L1, L2=L2, TPAD=TPAD, overflow=overflow)


def build_x0(r_slice, colidx):
    """Host-side iteration-0 gather: x0[(j,b), (mm,k)] = r[b, col] (pads BIGX)."""
    rpad = np.concatenate([r_slice, np.full((B, 1), BIGX, np.float32)], axis=1)
    x0 = rpad[:, colidx]                      # [B, NBLK, RPB, KPAD]
    x0 = x0.transpose(1, 0, 2, 3).reshape(P, EPB)
    return np.ascontiguousarray(x0)


# ---------------------------------------------------------------- kernel ----
def hoist_waits(nc, max_embedded=1):
    """Split multi-wait instructions into standalone EventSemaphore waits."""
    k = 0
    for f in nc.m.functions:
        for b in f.blocks:
            insts = b.instructions
            out = []
            for i in insts:
                tname = type(i).__name__
                si = i.sync_info
                if (si is not None and tname != "InstEventSemaphore"
                        and len(si.on_wait) > max_embedded):
                    waits = list(si.on_wait)
                    keep = waits[:max_embedded]
                    for w in waits[max_embedded:]:
                        es = mybir.InstEventSemaphore(
                            name=f"hoistw{k}", ins=[], outs=[])
                        k += 1
                        es.engine = i.engine
                        es.sync_info = mybir.SyncInfo(on_wait=[w], on_update=[])
                        nc.inst_map[es.name] = es
                        out.append(es)
                    i.sync_info = mybir.SyncInfo(
                        on_wait=keep, on_update=list(si.on_update))
                out.append(i)
            b.instructions = out


def build_bass(alpha, beta, tables):
    """alpha/beta: lists of 3 floats (baked as immediates)."""
    L1, L2, TPAD = tables["L1"], tables["L2"], tables["TPAD"]
    overflow = tables["overflow"]
    nc = bass.Bass("TRN2", target_bir_lowering=False, debug=False)
    r_d = nc.dram_tensor("r", [B, N], FP32, kind="ExternalInput")
    x0_d = nc.dram_tensor("x0", [P, EPB], FP32, kind="ExternalInput")
    zidx_d = nc.dram_tensor("zidx", [P, EPB // 16], U16, kind="ExternalInput")
    gidx_d = nc.dram_tensor("gidx", [P, TPAD // 16], U16, kind="ExternalInput")
    wmat_d = nc.dram_tensor("wmat", [P, P], FP16, kind="ExternalInput")
    out_d = nc.dram_tensor("out", [B, N], FP32, kind="ExternalOutput")
    HALF = N // 2

    with tile.TileContext(nc) as tc:
        with ExitStack() as ctx:
            pool = ctx.enter_context(tc.tile_pool(name="main", bufs=1))
            pspool = ctx.enter_context(tc.tile_pool(name="ps", bufs=1, space="PSUM"))

            r_rep = pool.tile([P, N], FP32)
            zrep = pool.tile([P, N + 2], FP32)    # col N = BIGX pad
            zfin = pool.tile([P, N], FP32)
            X = pool.tile([P, EPB], FP32)
            Xg = pool.tile([P, EPB], FP32)
            A3 = pool.tile([P, EPB], FP32)
            E = pool.tile([P, EPB + 2], FP16)     # col EPB = zero slot
            sgn = pool.tile([P, EPB], FP16)
            S = pool.tile([P, EPB], FP16)         # sign prefix products
            rsb = pool.tile([P, RPB + 1], FP16)   # col 0 = 1.0
            rs = pool.tile([P, RPB], FP16)
            sgf = pool.tile([P, EPB], FP16)       # sign(X)*rowsign
            S1 = pool.tile([P, EPB + 2], FP32)    # prefix-incl min (col0=BIG)
            S2 = pool.tile([P, EPB + 2], FP32)    # suffix-incl min
            Emin = pool.tile([P, EPB], FP32)      # exclude-self row min
            G3e = pool.tile([P, EPB], FP16)       # relu(al*Emin - al*beta)
            maskF = pool.tile([P, EPB], FP32)     # scan reset: BIG at k%16==15
            maskR = pool.tile([P, EPB], FP32)     # scan reset: BIG at k%16==0
            negb = pool.tile([P, EPB], FP16)      # sign-scan op1 filler (-1)
            G = pool.tile([P, TPAD], FP16)
            zidx = pool.tile([P, EPB // 16], U16)
            gidx = pool.tile([P, TPAD // 16], U16)
            wmat = pool.tile([P, P], FP16)
            biasc = pool.tile([P, ITERS], FP32)   # -alpha*beta per iter
            # one full bank per quarter so matmul groups don't share banks
            zpsb = [pspool.tile([P, 512], FP32, name=f"zps{q}")
                    for q in range(4)]

            # ---- static loads (x0/r on the HWDGE queues first; index
            # tables + wmat via the Pool SWDGE path, which is idle early) ----
            r_bc = bass.AP(tensor=r_d.ap().tensor, offset=0,
                           ap=[[0, NBLK], [N, B], [1, N]])
            nc.sync.dma_start(X[:], x0_d[:])
            nc.scalar.dma_start(r_rep[:], r_bc)
            nc.gpsimd.dma_start(zidx[:], zidx_d[:])
            nc.gpsimd.dma_start(gidx[:], gidx_d[:])
            nc.gpsimd.dma_start(wmat[:], wmat_d[:])
            nc.vector.memset(zrep[:, N:N + 2], float(BIGX))
            nc.vector.memset(E[:, EPB:EPB + 2], 0.0)
            nc.vector.memset(rsb[:, 0:1], 1.0)
            nc.vector.memset(negb[:], -1.0)
            nc.vector.memset(S1[:, 0:1], float(BIGX))
            nc.vector.memset(maskF[:], -1.0)
            nc.vector.memset(maskR[:], -1.0)
            mF3 = maskF[:].rearrange("p (a b) -> p a b", a=RPB)
            mR3 = maskR[:].rearrange("p (a b) -> p a b", a=RPB)
            nc.vector.memset(mF3[:, :, 15:16], float(BIGX))
            nc.vector.memset(mR3[:, :, 0:1], float(BIGX))
            for it in range(ITERS):
                nc.vector.memset(biasc[:, it:it + 1],
                                 -float(alpha[it]) * float(beta[it]))
            # consume the index-table DMA deps early + warm PE path
            idxtouch = pool.tile([P, 2], U16)
            nc.gpsimd.tensor_copy(idxtouch[:, 0:1], zidx[:, 0:1])
            nc.gpsimd.tensor_copy(idxtouch[:, 1:2], gidx[:, 0:1])
            nc.tensor.matmul(zpsb[0][0:1, 0:1], lhsT=wmat[0:B, 0:1],
                             rhs=wmat[0:B, 0:1], start=True, stop=True)

            def seg(t):   # [P, EPB] tile or AP -> [P, RPB, KPAD]
                ap = t if isinstance(t, bass.AP) else t[:]
                return ap.rearrange("p (a b) -> p a b", a=RPB)

            def rb(v):    # [P, RPB] row vec -> [P, RPB, KPAD] broadcast
                return v.unsqueeze(2).broadcast_to([P, RPB, KPAD])

            # overflow edges grouped by quarter (q = pos//144)
            ovf_by_q = ([], [], [], [])
            for (j0, p0, e0) in overflow:
                ovf_by_q[p0 // 144].append((j0, p0, e0))

            for it in range(ITERS):
                al = float(alpha[it])
                bias_ap = biasc[:, it:it + 1]
                last = it == ITERS - 1

                # ---- X = gather(zrep) - E  (it 0: host-precomputed X0) ----
                if it > 0:
                    nc.gpsimd.indirect_copy(Xg[:], zrep[:, 0:N + 1], zidx[:], True)
                    # strided over the 15 real slots; X pads keep BIGX from x0
                    nc.vector.tensor_sub(seg(X)[:, :, 0:ROW_DEG],
                                         seg(Xg)[:, :, 0:ROW_DEG],
                                         seg(E[:, 0:EPB])[:, :, 0:ROW_DEG])
                # sgn first: it heads the longest cross-engine chain
                # (ACT sgn -> Pool scan -> rowsign -> sgn_eff -> E)
                nc.scalar.activation(sgn[:], X[:],
                                     func=mybir.ActivationFunctionType.Sign)
                # A3 = |X| on DVE: (X * -1) max X
                nc.vector.scalar_tensor_tensor(A3[:], X[:], -1.0, X[:],
                                               op0=mybir.AluOpType.mult,
                                               op1=mybir.AluOpType.max)
                # prefix/suffix running-min scans; data1 holds BIG at each
                # row's pad slot, so `max` resets the running min per row
                nc.vector.tensor_tensor_scan(S1[:, 1:EPB + 1], A3[:],
                                             maskF[:], float(BIGX),
                                             op0=mybir.AluOpType.min,
                                             op1=mybir.AluOpType.max)
                nc.vector.tensor_tensor_scan(S2[:, 0:EPB][:, ::-1],
                                             A3[:][:, ::-1],
                                             maskR[:], float(BIGX),
                                             op0=mybir.AluOpType.min,
                                             op1=mybir.AluOpType.max)
                # exclude-self min: prefix before k, suffix after k
                nc.vector.tensor_tensor(Emin[:], S1[:, 0:EPB],
                                        S2[:, 1:EPB + 1],
                                        op=mybir.AluOpType.min)
                nc.scalar.activation(G3e[:], Emin[:],
                                     func=mybir.ActivationFunctionType.Relu,
                                     scale=al, bias=bias_ap)

                # ---- row sign parity on Pool ----
                nc.gpsimd.tensor_tensor_scan(S[:], sgn[:], negb[:], 1.0,
                                             op0=mybir.AluOpType.mult,
                                             op1=mybir.AluOpType.max)
                ends = seg(S)[:, :, 15:16]                  # [P, RPB, 1]
                nc.gpsimd.tensor_copy(rsb[:, 1:RPB + 1], ends)
                nc.gpsimd.tensor_mul(rs[:], rsb[:, 1:RPB + 1], rsb[:, 0:RPB])
                nc.gpsimd.tensor_mul(seg(sgf), seg(sgn), rb(rs[:]))

                # ---- E = relu(al*Emin - al*beta) * sgn * rowsign ----
                nc.vector.tensor_mul(E[:, 0:EPB], G3e[:], sgf[:])

                # ---- colsum: one gather + prefix adds + PE + z ----
                nc.gpsimd.indirect_copy(G[:], E[:, 0:EPB + 1], gidx[:], True)
                # add1 split so PE quarters start early (pi is depth-sorted)
                if L1 > HALF:
                    nc.vector.tensor_add(G[:, HALF:L1], G[:, HALF:L1],
                                         G[:, 576 + HALF:576 + L1])
                    nc.vector.tensor_add(G[:, 0:HALF], G[:, 0:HALF],
                                         G[:, 576:576 + HALF])
                else:
                    nc.vector.tensor_add(G[:, 0:L1], G[:, 0:L1],
                                         G[:, 576:576 + L1])
                if L2 > 0:
                    nc.vector.tensor_add(G[:, 0:L2], G[:, 0:L2],
                                         G[:, 576 + L1:576 + L1 + L2])

                # PE: 4 independent quarter groups (q3 first: clean region),
                # per-quarter z-add pipelined right after each group's stop
                for q in (3, 2, 1, 0):
                    ovf = ovf_by_q[q]
                    nc.tensor.matmul(zpsb[q][:, 0:144], lhsT=wmat[:],
                                     rhs=G[:, q * 144:q * 144 + 144],
                                     start=True, stop=(len(ovf) == 0))
                    for i, (j0, p0, e0) in enumerate(ovf):
                        nn = p0 - q * 144
                        nc.tensor.matmul(zpsb[q][:, nn:nn + 1],
                                         lhsT=wmat[32 * j0:32 * (j0 + 1), :],
                                         rhs=E[32 * j0:32 * (j0 + 1), e0:e0 + 1],
                                         start=False, stop=(i == len(ovf) - 1),
                                         tile_position=(32 * j0, 0))
                    sl = slice(q * 144, (q + 1) * 144)
                    if not last:
                        nc.vector.tensor_add(zrep[:, sl], zpsb[q][:, 0:144],
                                             r_rep[:, sl])
                    else:
                        nc.vector.tensor_add(zfin[0:B, sl],
                                             zpsb[q][0:B, 0:144],
                                             r_rep[0:B, sl])
            nc.sync.dma_start(out_d[:], zfin[0:B, :])

    hoist_waits(nc)
    return nc


# ------------------------------------------------------------ host driver ----
_CACHE = {}


def kernel(r, H, alpha, beta):
    r = np.asarray(r, dtype=np.float32)
    H = np.asarray(H, dtype=np.float32)
    alpha_l = [float(x) for x in np.asarray(alpha).reshape(-1)]
    beta_l = [float(x) for x in np.asarray(beta).reshape(-1)]

    key = (H.tobytes(), tuple(alpha_l), tuple(beta_l))
    if key not in _CACHE:
        tables = build_tables(H)
        nc = build_bass(alpha_l, beta_l, tables)
        _CACHE[key] = (tables, nc)
    tables, nc = _CACHE[key]
    porder = tables["porder"]

    from concourse.bass_utils import run_bass_kernel_spmd
    in_maps = []
    for c in range(8):
        rs_ = np.ascontiguousarray(r[c * B:(c + 1) * B])
        in_maps.append({
            "r": np.ascontiguousarray(rs_[:, porder]),
            "x0": build_x0(rs_, tables["colidx"]),
            "zidx": tables["zidx"],
            "gidx": tables["gidx"],
            "wmat": tables["wmat"],
        })
    # the first execution on a freshly-attached device occasionally fails
    # with NRT_EXEC_UNIT_UNRECOVERABLE; a retry succeeds
    last = None
    for _attempt in range(3):
        try:
            res = run_bass_kernel_spmd(nc, in_maps, core_ids=list(range(8)))
            break
        except Exception as e:  # noqa: BLE001
            last = e
    else:
        raise last
    out_p = np.concatenate([res.results[c]["out"] for c in range(8)], axis=0)
    out = np.empty_like(out_p)
    out[:, porder] = out_p
    return out.astype(np.float32)


# iteration 0 skips the X-gather: X comes from the x0 DMA issued at startup.
# The gather at the END of iterations 0 and 1 prepares the next X.


# revision 29
# speedup vs baseline: 1.0351x; 1.0295x over previous
"""TRN2 Bass kernel for nn_NMS (offset min-sum LDPC decoder, batch 256).

Self-contained: derives all index tables from the H input at call time,
shards the batch across 8 NeuronCores (32 per core), runs one SPMD Bass
program via run_bass_kernel_spmd, and gathers the full [256, 576] output.

Per-core layout: 128 partitions = 4 row-blocks x 32 batch; each row-block's
edges live on the free axis as [36 rows x 16 slots] (15 real + 1 pad).

v4 pipeline (per decoding iteration):
  X = gather(zrep) - E                 (Pool gather; DVE sub; fp32 X)
  A3 = |X|                             (DVE stt: (X*-1) max X)
  G3 = relu(al*|X| - al*beta)          (ACT, fp16 - the min-sum magnitude
                                        transform, applied BEFORE the min:
                                        it is monotone, so min(g(x)) = g(min x)
                                        exactly, even after fp16 rounding)
  exclude-self row min via two scans   (DVE tensor_tensor_scan over G3,
                                        op0=min op1=max; a BIG in data1 at
                                        each row-pad slot resets the running
                                        min - no min1/min2/argmin machinery)
    Gex = min(prefix_excl, suffix_excl)  (fp16 packed TT of shifted views;
                                        the suffix scan runs on reversed APs)
  rowsign = segmented reduce(mult)     (DVE, over sign(X) from ACT)
  E   = Gex * (sign(X)*rowsign)        (sgf on Pool; fp16 packed TT, 2x)
  colsum via ONE permuted-run gather   (Pool) + 2 prefix adds (fp16 DVE)
  cross-block sum + 4x replicate       (PE one-hot fp16 matmul; depth>=4
                                        edges via tiny accumulate-matmuls)
  Z = colsum + r                       (DVE, fp32)
Columns are globally permuted (descending max-per-block degree) so the
colsum gather is one instruction with prefix-aligned depth runs; the host
permutes r on the way in and un-permutes the output.

Accuracy: X/Z stay fp32 and the exclude-min is exact (min commutes with
the monotone fp16 transform); E/colsum are fp16 (measured end-to-end rel
err ~1e-4 vs the fp32 reference).

Multi-wait instructions are post-processed into standalone EventSemaphore
waits (hoist_waits) because this walrus build accepts only one sync-wait
slot per TPB instruction.
"""
import numpy as np
from contextlib import ExitStack

import concourse.bass as bass
import concourse.tile as tile
from concourse import mybir, library_config

FP32 = mybir.dt.float32
FP16 = mybir.dt.float16
U16 = mybir.dt.uint16

P = 128
B = 32           # batch per core
NBLK = 4
RPB = 36         # rows per block
KPAD = 16        # padded row degree
ROW_DEG = 15
EPB = RPB * KPAD  # 576 edge slots per block
N = 576          # columns
D_KEEP = 3       # depth runs gathered; deeper edges via tiny accum-matmuls
ITERS = 3
BIGX = np.float32(30000.0)   # pad value for X/zrep (fp16-safe after *alpha)


# ---------------------------------------------------------------- tables ----
def build_tables(H):
    MROWS = H.shape[0]
    cols = np.array([np.nonzero(H[m])[0] for m in range(MROWS)], dtype=np.int64)
    assert cols.shape == (MROWS, ROW_DEG)

    # ---- block assignment: minimize (depth>=4 edges, L2, L1) via convex
    # per-(block,col) penalty, delta-evaluated row swaps ----
    PEN = np.array([0.0, 0.0, 1.0, 60.0, 4000.0, 3e5, 2e7, 1e9, 1e9],
                   dtype=np.float64)

    def metrics(cnt):
        mx = cnt.max(axis=0)
        return (int(np.maximum(cnt - 3, 0).sum()), int((mx >= 3).sum()),
                int((mx >= 2).sum()), int(cnt.max()))

    best = None
    for restart in range(2):
        rs = np.random.default_rng(restart)
        perm = rs.permutation(MROWS)
        assign = np.zeros(MROWS, dtype=np.int64)
        sizes = [0] * NBLK
        cnt = np.zeros((NBLK, N), dtype=np.int32)
        for m in perm:
            bestj, bestpen = None, None
            for j in range(NBLK):
                if sizes[j] >= RPB:
                    continue
                p = PEN[cnt[j, cols[m]] + 1].sum()
                if bestpen is None or p < bestpen:
                    bestj, bestpen = j, p
            assign[m] = bestj
            sizes[bestj] += 1
            cnt[bestj, cols[m]] += 1
        for _sweep in range(40):
            improved = False
            for m1 in range(MROWS):
                for m2 in range(m1 + 1, MROWS):
                    j1, j2 = assign[m1], assign[m2]
                    if j1 == j2:
                        continue
                    c1, c2 = cols[m1], cols[m2]
                    cn1, cn2 = cnt[j1], cnt[j2]
                    aff1, aff2 = {}, {}
                    for c in c1:
                        aff1[c] = aff1.get(c, 0) - 1
                        aff2[c] = aff2.get(c, 0) + 1
                    for c in c2:
                        aff1[c] = aff1.get(c, 0) + 1
                        aff2[c] = aff2.get(c, 0) - 1
                    d = 0.0
                    for c, dd in aff1.items():
                        d += PEN[cn1[c] + dd] - PEN[cn1[c]]
                    for c, dd in aff2.items():
                        d += PEN[cn2[c] + dd] - PEN[cn2[c]]
                    if d < -1e-9:
                        for c, dd in aff1.items():
                            cn1[c] += dd
                        for c, dd in aff2.items():
                            cn2[c] += dd
                        assign[m1], assign[m2] = j2, j1
                        improved = True
            if not improved:
                break
        met = metrics(cnt)
        if best is None or met[:3] < best[0][:3]:
            best = (met, assign.copy(), cnt.copy())
    met, assign, cnt = best
    assert cnt.max() <= 4, f"block depth {cnt.max()} > 4"

    colidx = np.full((NBLK, RPB, KPAD), N, dtype=np.int64)
    rows_of_block = [np.array([m for m in range(MROWS) if assign[m] == j],
                              dtype=np.int64) for j in range(NBLK)]
    for j in range(NBLK):
        for mm, m in enumerate(rows_of_block[j]):
            colidx[j, mm, :ROW_DEG] = cols[m]

    # per-(block, col, depth) edge positions
    strip_pos = np.full((NBLK, N, 4), EPB, dtype=np.int64)
    fill = np.zeros((NBLK, N), dtype=np.int64)
    for j in range(NBLK):
        for mm in range(RPB):
            for k in range(ROW_DEG):
                n = colidx[j, mm, k]
                d = fill[j, n]
                fill[j, n] = d + 1
                strip_pos[j, n, d] = mm * KPAD + k

    # global column permutation: descending capped max-depth -> prefix runs
    mdeg = np.minimum(cnt, D_KEEP).max(axis=0)          # [N], 0..3
    porder = np.argsort(-mdeg, kind="stable").astype(np.int64)
    pos = np.zeros(N, dtype=np.int64)
    pos[porder] = np.arange(N)
    L1 = int((mdeg >= 2).sum())
    L2 = int((mdeg >= 3).sum())

    # overflow: per-block depth-3 edges (cnt==4) -> accumulate-matmuls
    overflow = [(j, int(pos[n]), int(strip_pos[j, n, 3]))
                for j in range(NBLK) for n in range(N)
                if strip_pos[j, n, 3] != EPB]
    assert len(overflow) <= 16, f"too many overflow edges: {len(overflow)}"

    # gather table: [run0: 576][run1: L1][run2: L2][pad to %16]
    T = 576 + L1 + L2
    TPAD = (T + 15) // 16 * 16
    gvals = []
    for j in range(NBLK):
        v = np.full(TPAD, EPB, dtype=np.int64)
        i = 0
        for d in range(D_KEEP):
            lim = [576, L1, L2][d]
            for p_ in range(lim):
                v[i] = strip_pos[j, porder[p_], d]
                i += 1
        gvals.append(v)

    zvals = []
    for j in range(NBLK):
        v = np.empty(EPB, dtype=np.int64)
        flat = colidx[j].reshape(-1)
        for i in range(EPB):
            v[i] = pos[flat[i]] if flat[i] < N else N
        zvals.append(v)

    def wrap(vals_per_block, num_idxs):
        t = np.zeros((P, num_idxs // 16), dtype=np.uint16)
        for c in range(8):
            j = c // 2
            v = vals_per_block[j]
            for i in range(num_idxs):
                t[16 * c + i % 16, i // 16] = v[i]
        return t

    zidx = wrap(zvals, EPB)
    gidx = wrap(gvals, TPAD)

    # one-hot cross-block sum + replicate: W[(j',b'), (j,b)] = (b'==b)
    wmat = np.zeros((P, P), dtype=np.float16)
    for jp in range(NBLK):
        for bp in range(B):
            for j in range(NBLK):
                wmat[jp * B + bp, j * B + bp] = 1.0
    return dict(zidx=zidx, gidx=gidx, wmat=wmat, colidx=colidx,
                porder=porder, L1=L1, L2=L2, TPAD=TPAD, overflow=overflow)


def build_x0(r_slice, colidx):
    """Host-side iteration-0 gather: x0[(j,b), (mm,k)] = r[b, col] (pads BIGX)."""
    rpad = np.concatenate([r_slice, np.full((B, 1), BIGX, np.float32)], axis=1)
    x0 = rpad[:, colidx]                      # [B, NBLK, RPB, KPAD]
    x0 = x0.transpose(1, 0, 2, 3).reshape(P, EPB)
    return np.ascontiguousarray(x0)


# ---------------------------------------------------------------- kernel ----
def hoist_waits(nc, max_embedded=1):
    """Split multi-wait instructions into standalone EventSemaphore waits."""
    k = 0
    for f in nc.m.functions:
        for b in f.blocks:
            insts = b.instructions
            out = []
            for i in insts:
                tname = type(i).__name__
                si = i.sync_info
                if (si is not None and tname != "InstEventSemaphore"
                        and len(si.on_wait) > max_embedded):
                    waits = list(si.on_wait)
                    keep = waits[:max_embedded]
                    for w in waits[max_embedded:]:
                        es = mybir.InstEventSemaphore(
                            name=f"hoistw{k}", ins=[], outs=[])
                        k += 1
                        es.engine = i.engine
                        es.sync_info = mybir.SyncInfo(on_wait=[w], on_update=[])
                        nc.inst_map[es.name] = es
                        out.append(es)
                    i.sync_info = mybir.SyncInfo(
                        on_wait=keep, on_update=list(si.on_update))
                out.append(i)
            b.instructions = out


def build_bass(alpha, beta, tables):
    """alpha/beta: lists of 3 floats (baked as immediates)."""
    L1, L2, TPAD = tables["L1"], tables["L2"], tables["TPAD"]
    overflow = tables["overflow"]
    nc = bass.Bass("TRN2", target_bir_lowering=False, debug=False)
    r_d = nc.dram_tensor("r", [B, N], FP32, kind="ExternalInput")
    x0_d = nc.dram_tensor("x0", [P, EPB], FP32, kind="ExternalInput")
    zidx_d = nc.dram_tensor("zidx", [P, EPB // 16], U16, kind="ExternalInput")
    gidx_d = nc.dram_tensor("gidx", [P, TPAD // 16], U16, kind="ExternalInput")
    wmat_d = nc.dram_tensor("wmat", [P, P], FP16, kind="ExternalInput")
    out_d = nc.dram_tensor("out", [B, N], FP32, kind="ExternalOutput")
    HALF = N // 2

    with tile.TileContext(nc) as tc:
        with ExitStack() as ctx:
            pool = ctx.enter_context(tc.tile_pool(name="main", bufs=1))
            pspool = ctx.enter_context(tc.tile_pool(name="ps", bufs=1, space="PSUM"))

            r_rep = pool.tile([P, N], FP32)
            zrep = pool.tile([P, N + 2], FP32)    # col N = BIGX pad
            zfin = pool.tile([P, N], FP32)
            X = pool.tile([P, EPB], FP32)
            Xg = pool.tile([P, EPB], FP32)
            A3 = pool.tile([P, EPB], FP32)
            E = pool.tile([P, EPB + 2], FP16)     # col EPB = zero slot
            sgn = pool.tile([P, EPB], FP16)
            rs = pool.tile([P, RPB], FP16)
            sgf = pool.tile([P, EPB], FP16)       # sign(X)*rowsign
            G3 = pool.tile([P, EPB], FP16)        # relu(al*|X| - al*beta)
            S1 = pool.tile([P, EPB + 2], FP16)    # prefix-incl min (col0=BIG)
            S2 = pool.tile([P, EPB + 2], FP16)    # suffix-incl min
            Gex = pool.tile([P, EPB], FP16)       # exclude-self row min of G3
            maskF = pool.tile([P, EPB], FP16)     # scan reset: BIG at k%16==15
            maskR = pool.tile([P, EPB], FP16)     # scan reset: BIG at k%16==0
            G = pool.tile([P, TPAD], FP16)
            zidx = pool.tile([P, EPB // 16], U16)
            gidx = pool.tile([P, TPAD // 16], U16)
            wmat = pool.tile([P, P], FP16)
            biasc = pool.tile([P, ITERS], FP32)   # -alpha*beta per iter
            # one full bank per quarter so matmul groups don't share banks
            zpsb = [pspool.tile([P, 512], FP32, name=f"zps{q}")
                    for q in range(4)]

            # ---- static loads (x0/r on the HWDGE queues first; index
            # tables + wmat via the Pool SWDGE path, which is idle early) ----
            r_bc = bass.AP(tensor=r_d.ap().tensor, offset=0,
                           ap=[[0, NBLK], [N, B], [1, N]])
            nc.sync.dma_start(X[:], x0_d[:])
            nc.scalar.dma_start(r_rep[:], r_bc)
            nc.gpsimd.dma_start(zidx[:], zidx_d[:])
            nc.gpsimd.dma_start(gidx[:], gidx_d[:])
            nc.gpsimd.dma_start(wmat[:], wmat_d[:])
            nc.vector.memset(zrep[:, N:N + 2], float(BIGX))
            nc.vector.memset(E[:, EPB:EPB + 2], 0.0)
            nc.vector.memset(S1[:, 0:1], float(BIGX))
            nc.vector.memset(maskF[:], -1.0)
            nc.vector.memset(maskR[:], -1.0)
            mF3 = maskF[:].rearrange("p (a b) -> p a b", a=RPB)
            mR3 = maskR[:].rearrange("p (a b) -> p a b", a=RPB)
            nc.vector.memset(mF3[:, :, 15:16], float(BIGX))
            nc.vector.memset(mR3[:, :, 0:1], float(BIGX))
            # A3 pads never rewritten (strided stt) - keep them at BIGX
            A33 = A3[:].rearrange("p (a b) -> p a b", a=RPB)
            nc.vector.memset(A33[:, :, 15:16], float(BIGX))
            for it in range(ITERS):
                nc.vector.memset(biasc[:, it:it + 1],
                                 -float(alpha[it]) * float(beta[it]))
            # consume the index-table DMA deps early + warm PE path
            idxtouch = pool.tile([P, 2], U16)
            nc.gpsimd.tensor_copy(idxtouch[:, 0:1], zidx[:, 0:1])
            nc.gpsimd.tensor_copy(idxtouch[:, 1:2], gidx[:, 0:1])
            nc.tensor.matmul(zpsb[0][0:1, 0:1], lhsT=wmat[0:B, 0:1],
                             rhs=wmat[0:B, 0:1], start=True, stop=True)

            def seg(t):   # [P, EPB] tile or AP -> [P, RPB, KPAD]
                ap = t if isinstance(t, bass.AP) else t[:]
                return ap.rearrange("p (a b) -> p a b", a=RPB)

            def rb(v):    # [P, RPB] row vec -> [P, RPB, KPAD] broadcast
                return v.unsqueeze(2).broadcast_to([P, RPB, KPAD])

            # overflow edges grouped by quarter (q = pos//144)
            ovf_by_q = ([], [], [], [])
            for (j0, p0, e0) in overflow:
                ovf_by_q[p0 // 144].append((j0, p0, e0))

            for it in range(ITERS):
                al = float(alpha[it])
                bias_ap = biasc[:, it:it + 1]
                last = it == ITERS - 1

                # ---- X = gather(zrep) - E  (it 0: host-precomputed X0) ----
                if it > 0:
                    nc.gpsimd.indirect_copy(Xg[:], zrep[:, 0:N + 1], zidx[:], True)
                    # strided over the 15 real slots; X pads keep BIGX from x0
                    nc.vector.tensor_sub(seg(X)[:, :, 0:ROW_DEG],
                                         seg(Xg)[:, :, 0:ROW_DEG],
                                         seg(E[:, 0:EPB])[:, :, 0:ROW_DEG])
                # sgn first on ACT: it heads the rowsign/sgf chain
                nc.scalar.activation(sgn[:], X[:],
                                     func=mybir.ActivationFunctionType.Sign)
                # A3 = |X| on DVE: (X * -1) max X  (pads stay BIGX)
                nc.vector.scalar_tensor_tensor(seg(A3)[:, :, 0:ROW_DEG],
                                               seg(X)[:, :, 0:ROW_DEG], -1.0,
                                               seg(X)[:, :, 0:ROW_DEG],
                                               op0=mybir.AluOpType.mult,
                                               op1=mybir.AluOpType.max)
                nc.scalar.activation(G3[:], A3[:],
                                     func=mybir.ActivationFunctionType.Relu,
                                     scale=al, bias=bias_ap)
                # rowsign = product of signs per row (pads are +1)
                nc.vector.tensor_reduce(rs[:], seg(sgn)[:, :, 0:ROW_DEG],
                                        axis=mybir.AxisListType.X,
                                        op=mybir.AluOpType.mult)
                # prefix/suffix running-min scans over G3; data1 holds BIG at
                # each row's pad slot, so `max` resets the running min per row
                nc.vector.tensor_tensor_scan(S1[:, 1:EPB + 1], G3[:],
                                             maskF[:], float(BIGX),
                                             op0=mybir.AluOpType.min,
                                             op1=mybir.AluOpType.max)
                nc.vector.tensor_tensor_scan(S2[:, 0:EPB][:, ::-1],
                                             G3[:][:, ::-1],
                                             maskR[:], float(BIGX),
                                             op0=mybir.AluOpType.min,
                                             op1=mybir.AluOpType.max)
                # exclude-self min: prefix before k, suffix after k (packed)
                nc.vector.tensor_tensor(Gex[:], S1[:, 0:EPB],
                                        S2[:, 1:EPB + 1],
                                        op=mybir.AluOpType.min)

                # ---- sgf = sign * rowsign on Pool; E = Gex * sgf ----
                nc.gpsimd.tensor_mul(seg(sgf), seg(sgn), rb(rs[:]))
                nc.vector.tensor_mul(E[:, 0:EPB], Gex[:], sgf[:])

                # ---- colsum: one gather + prefix adds + PE + z ----
                nc.gpsimd.indirect_copy(G[:], E[:, 0:EPB + 1], gidx[:], True)
                # add1 split so PE quarters start early (pi is depth-sorted)
                if L1 > HALF:
                    nc.vector.tensor_add(G[:, HALF:L1], G[:, HALF:L1],
                                         G[:, 576 + HALF:576 + L1])
                    nc.vector.tensor_add(G[:, 0:HALF], G[:, 0:HALF],
                                         G[:, 576:576 + HALF])
                else:
                    nc.vector.tensor_add(G[:, 0:L1], G[:, 0:L1],
                                         G[:, 576:576 + L1])
                if L2 > 0:
                    nc.vector.tensor_add(G[:, 0:L2], G[:, 0:L2],
                                         G[:, 576 + L1:576 + L1 + L2])

                # PE: 4 independent quarter groups (q3 first: clean region),
                # per-quarter z-add pipelined right after each group's stop
                for q in (3, 2, 1, 0):
                    ovf = ovf_by_q[q]
                    nc.tensor.matmul(zpsb[q][:, 0:144], lhsT=wmat[:],
                                     rhs=G[:, q * 144:q * 144 + 144],
                                     start=True, stop=(len(ovf) == 0))
                    for i, (j0, p0, e0) in enumerate(ovf):
                        nn = p0 - q * 144
                        nc.tensor.matmul(zpsb[q][:, nn:nn + 1],
                                         lhsT=wmat[32 * j0:32 * (j0 + 1), :],
                                         rhs=E[32 * j0:32 * (j0 + 1), e0:e0 + 1],
                                         start=False, stop=(i == len(ovf) - 1),
                                         tile_position=(32 * j0, 0))
                    sl = slice(q * 144, (q + 1) * 144)
                    if not last:
                        nc.vector.tensor_add(zrep[:, sl], zpsb[q][:, 0:144],
                                             r_rep[:, sl])
                    else:
                        nc.vector.tensor_add(zfin[0:B, sl],
                                             zpsb[q][0:B, 0:144],
                                             r_rep[0:B, sl])
            nc.sync.dma_start(out_d[:], zfin[0:B, :])

    hoist_waits(nc)
    return nc


# ------------------------------------------------------------ host driver ----
_CACHE = {}


def kernel(r, H, alpha, beta):
    r = np.asarray(r, dtype=np.float32)
    H = np.asarray(H, dtype=np.float32)
    alpha_l = [float(x) for x in np.asarray(alpha).reshape(-1)]
    beta_l = [float(x) for x in np.asarray(beta).reshape(-1)]

    key = (H.tobytes(), tuple(alpha_l), tuple(beta_l))
    if key not in _CACHE:
        tables = build_tables(H)
        nc = build_bass(alpha_l, beta_l, tables)
        _CACHE[key] = (tables, nc)
    tables, nc = _CACHE[key]
    porder = tables["porder"]

    from concourse.bass_utils import run_bass_kernel_spmd
    in_maps = []
    for c in range(8):
        rs_ = np.ascontiguousarray(r[c * B:(c + 1) * B])
        in_maps.append({
            "r": np.ascontiguousarray(rs_[:, porder]),
            "x0": build_x0(rs_, tables["colidx"]),
            "zidx": tables["zidx"],
            "gidx": tables["gidx"],
            "wmat": tables["wmat"],
        })
    # the first execution on a freshly-attached device occasionally fails
    # with NRT_EXEC_UNIT_UNRECOVERABLE; a retry succeeds
    last = None
    for _attempt in range(3):
        try:
            res = run_bass_kernel_spmd(nc, in_maps, core_ids=list(range(8)))
            break
        except Exception as e:  # noqa: BLE001
            last = e
    else:
        raise last
    out_p = np.concatenate([res.results[c]["out"] for c in range(8)], axis=0)
    out = np.empty_like(out_p)
    out[:, porder] = out_p
    return out.astype(np.float32)


# iteration 0 skips the X-gather: X comes from the x0 DMA issued at startup.
# The gather at the END of iterations 0 and 1 prepares the next X.
